# revision 11
# baseline (speedup 1.0000x reference)
"""NT-Xent (SimCLR) loss kernel for Trainium2, 8 NeuronCores, row-parallel.

Math (reference): z = concat(zA, zB) [N=8192, D=256]; zn = z / ||z||;
sim = zn @ zn.T / T (T=0.5); per_row i = logsumexp_{j != i}(sim[i, :]) -
sim[i, (i+B) % N]; loss = sum(per_row) / N.

Key facts exploited:
  * sim in [-2, 2] (cosine / 0.5), so no max-subtraction is needed for a
    stable logsumexp: sum exp(sim) in fp32 directly.
  * sim[i, i] = |zn_i|^2 / T ~= 2, so the diagonal mask is a constant
    subtraction of e^2 from the row sum (error ~e-5 relative, see below).
  * Each core's program is IDENTICAL: core c receives z rotated by c*1024
    rows, so its local rows are always columns [0, 1024) of its (rotated)
    Gram block, the self-diagonal is at j == i, and the positive partner is
    at j == i + 4096 — all offsets static.

Per-core device pipeline (Tile framework schedules engines):
  1. DMA zT (bf16, [2, 128, 8192]: D on partitions, 2 k-tiles) into SBUF.
  2. sq = zT*zT (DVE, bf16 2x); ones[128,128] @ sq (PE) accumulates the
     column sum-of-squares broadcast across all 128 partitions in PSUM.
  3. norm = sqrt(sumsq) (ACT); rinv = reciprocal_approx_fast(norm) (DVE,
     ~51 ULP); znT = zT * rinv (DVE) -> normalized, bf16.
  4. For each of 8 m-tiles (128 rows): Gram chunk G = znT_local.T @ znT
     (PE, bf16, fp32 PSUM accumulate over the 2 k-tiles), then ACT
     exp(2*G) with accum_out giving the row sums fused; the positive pair
     is pulled off the nb==2 chunk with an identity-masked
     tensor_tensor_reduce (pos = 2*G[i, i+4096]).
  5. Tail: per_row = Log(S - e^2) - pos (one ACT Log over [128, 8]).
Host: sums the 8 cores' [128, 8] per-row losses and divides by N.
"""

import numpy as np

N = 8192
D = 256
ROWS_PER_CORE = 1024
NCORES = 8
M_TILES = 8          # 1024 / 128 local row tiles
CHUNK = 2048         # column chunk (4 PSUM banks fp32)
NB = N // CHUNK      # 4 chunks
SUB = 512            # matmul moving free dim (1 PSUM bank fp32)
TEMP = 0.5
E2 = float(np.exp(np.float32(1.0 / TEMP)))

_NC_CACHE = {}

LAST_RESULTS = None


def _build_bass():
    import concourse.bacc as bacc
    import concourse.tile as tile
    from concourse import mybir

    f32 = mybir.dt.float32
    bf16 = mybir.dt.bfloat16
    AF = mybir.ActivationFunctionType
    ALU = mybir.AluOpType

    # Bacc (not raw Bass): its finalize() runs generate_event_semaphores /
    # move_matmul_waits_to_ldweights, which legalize multi-wait sync_info
    # for the TRN2 ISA (instructions can encode only 1-2 waits).
    nc = bacc.Bacc(None)
    zT_d = nc.dram_tensor("zT", [2, 128, N], bf16, kind="ExternalInput")
    ident_d = nc.dram_tensor("ident", [128, 128], f32, kind="ExternalInput")
    loss_d = nc.dram_tensor("loss", [128, M_TILES], f32, kind="ExternalOutput")

    with tile.TileContext(nc) as tc:
        with (
            tc.tile_pool(name="persist", bufs=1) as persist,
            tc.tile_pool(name="scratch", bufs=2) as scratch,
            tc.tile_pool(name="esc", bufs=3) as esc,
            tc.tile_pool(name="psum", bufs=2, space="PSUM") as psum,
        ):
            id_t = persist.tile([128, 128], f32, tag="ident")
            nc.sync.dma_start(out=id_t[:], in_=ident_d[:])
            # DVE-owned copy: raw-ISA TT ops (tensor_tensor_reduce) can only
            # encode few sync waits, so feed them from a same-engine tile.
            id_dve = persist.tile([128, 128], f32, tag="ident_dve")
            nc.vector.tensor_copy(id_dve[:], id_t[:])
            ones_t = persist.tile([128, 128], bf16, tag="ones")
            nc.vector.memset(ones_t[:], 1.0)

            zt = [
                [
                    persist.tile(
                        [128, CHUNK], bf16, tag=f"zt_{k}_{c}", name=f"zt_{k}_{c}"
                    )
                    for c in range(NB)
                ]
                for k in range(2)
            ]
            znT = [
                [
                    persist.tile(
                        [128, CHUNK], bf16, tag=f"znT_{k}_{c}", name=f"znT_{k}_{c}"
                    )
                    for c in range(NB)
                ]
                for k in range(2)
            ]
            for c in range(NB):
                for k in range(2):
                    nc.sync.dma_start(
                        out=zt[k][c][:], in_=zT_d[k, :, c * CHUNK : (c + 1) * CHUNK]
                    )

            # ---- prep: column norms (broadcast across partitions) + normalize
            for c in range(NB):
                # unique tiles (no slot reuse) keep WAR waits off the TT ops
                sq = [
                    scratch.tile(
                        [128, CHUNK], bf16, tag=f"sq{k}_{c}", name=f"sq{k}_{c}"
                    )
                    for k in range(2)
                ]
                for k in range(2):
                    nc.vector.tensor_mul(sq[k][:], zt[k][c][:], zt[k][c][:])
                ss = psum.tile([128, CHUNK], f32, tag="G")
                for k in range(2):
                    for s in range(CHUNK // SUB):
                        nc.tensor.matmul(
                            ss[:, s * SUB : (s + 1) * SUB],
                            ones_t[:],
                            sq[k][:, s * SUB : (s + 1) * SUB],
                            start=(k == 0),
                            stop=(k == 1),
                        )
                nrm = scratch.tile([128, CHUNK], f32, tag="nrm")
                nc.scalar.sqrt(nrm[:], ss[:])
                rinv = scratch.tile([128, CHUNK], f32, tag="rinv")
                nc.vector.reciprocal_approx_fast(out=rinv[:], in_=nrm[:])
                for k in range(2):
                    nc.vector.tensor_mul(znT[k][c][:], zt[k][c][:], rinv[:])

            Sall = persist.tile([128, M_TILES], f32, tag="Sall")
            posT = persist.tile([128, M_TILES], f32, tag="posT")
            edump = persist.tile([128, CHUNK], bf16, tag="edump")

            # ---- main: Gram row-block, exp, rowsum via DVE tensor_scalar
            # accumulate (ACT accum_out and tensor_tensor_reduce hang/fail on
            # this runtime, so neither is used)
            for t in range(M_TILES):
                S4 = scratch.tile([128, NB], f32, tag="S4")
                for c in range(NB):
                    G = psum.tile([128, CHUNK], f32, tag="G")
                    for k in range(2):
                        lhs = znT[k][0][:, t * 128 : (t + 1) * 128]
                        for s in range(CHUNK // SUB):
                            nc.tensor.matmul(
                                G[:, s * SUB : (s + 1) * SUB],
                                lhs,
                                znT[k][c][:, s * SUB : (s + 1) * SUB],
                                start=(k == 0),
                                stop=(k == 1),
                            )
                    if c == 2:
                        # partner cols [4096 + t*128, 4096 + (t+1)*128) live here
                        scr = scratch.tile(
                            [128, 128], f32, tag=f"posm{t}", name=f"posm{t}"
                        )
                        nc.vector.tensor_mul(
                            scr[:], G[:, t * 128 : t * 128 + 128], id_dve[:]
                        )
                        nc.vector.tensor_reduce(
                            out=posT[:, t : t + 1], in_=scr[:],
                            axis=mybir.AxisListType.X, op=ALU.add,
                        )
                    e = esc.tile([128, CHUNK], bf16, tag="esc")
                    nc.scalar.activation(
                        out=e[:], in_=G[:], func=AF.Exp, scale=float(1.0 / TEMP)
                    )
                    nc.vector.tensor_scalar(
                        out=edump[:], in0=e[:], scalar1=1.0, scalar2=0.0,
                        op0=ALU.mult, op1=ALU.add, accum_out=S4[:, c : c + 1],
                    )
                nc.vector.tensor_reduce(
                    out=Sall[:, t : t + 1], in_=S4[:], axis=mybir.AxisListType.X,
                    op=ALU.add,
                )

            # ---- tail: per_row = log(S - e^2) - 2*pos_G
            neg_e2 = persist.tile([128, 1], f32, tag="neg_e2")
            nc.vector.memset(neg_e2[:], float(-E2))
            lg = persist.tile([128, M_TILES], f32, tag="lg")
            nc.scalar.activation(
                out=lg[:], in_=Sall[:], func=AF.Ln, bias=neg_e2[:], scale=1.0
            )
            pos2 = persist.tile([128, M_TILES], f32, tag="pos2")
            nc.vector.tensor_scalar_mul(pos2[:], posT[:], float(1.0 / TEMP))
            loss_t = persist.tile([128, M_TILES], f32, tag="loss")
            nc.vector.tensor_sub(loss_t[:], lg[:], pos2[:])
            nc.sync.dma_start(out=loss_d[:], in_=loss_t[:])

    nc.finalize()  # Bacc.finalize -> compile(): sync-wait legalization etc.
    return nc


def _get_nc():
    if "nc" not in _NC_CACHE:
        _NC_CACHE["nc"] = _build_bass()
    return _NC_CACHE["nc"]


def kernel(zA, zB):
    global LAST_RESULTS
    from concourse import mybir
    from concourse.bass_utils import run_bass_kernel_spmd

    np_bf16 = mybir.dt.np(mybir.dt.bfloat16)

    zA = np.asarray(zA, dtype=np.float32)
    zB = np.asarray(zB, dtype=np.float32)
    z = np.concatenate([zA, zB], axis=0)          # [N, D]
    z16 = z.astype(np_bf16)
    ident = np.eye(128, dtype=np.float32)

    in_maps = []
    for c in range(NCORES):
        zr = np.roll(z16, -c * ROWS_PER_CORE, axis=0)         # rotate rows
        zTc = np.ascontiguousarray(zr.T).reshape(2, 128, N)   # [D, N] view
        in_maps.append({"zT": zTc, "ident": ident})

    nc = _get_nc()
    res = run_bass_kernel_spmd(nc, in_maps, list(range(NCORES)))
    LAST_RESULTS = res

    total = 0.0
    for r in res.results:
        total += float(r["loss"].astype(np.float64).sum())
    return np.float32(total / N)
